# revision 1
# baseline (speedup 1.0000x reference)
"""Trainium2 Bass kernel for nn_EquivBlock (GNN message passing).

Math (reference):
    h   = (x @ W.T + b) / 256            # [N, H] node projection
    phi = h[src] - h[dst]                # [E, H] per-edge message
    out = (v + u[:, :, None] * phi[:, None, :]) / 2

Device mapping (8 NeuronCores, SPMD, edges sharded):
    - every core computes the full h table (PE matmul over xT tiles,
      bias folded in with a K=1 accumulate matmul; W,b pre-scaled by
      1/256 on host) and writes it to a DRAM scratch table,
    - edges laid out partition-major (edge = p*COLS + j) so the big
      v/out streams move as 128 contiguous ~18KB descriptors per group,
    - per 128-edge tile-column: two indirect DMA gathers (h[src],
      h[dst]) into group tiles; per group of GK columns: one DVE
      subtract -> phi, one DVE broadcast-multiply by u (step-0 APs),
      one DVE add of the plain-DMA'd v block, one ScalarE *0.5, store.

Host side only shards/pads/relayouts inputs and folds the constant
1/256 into W and b.
"""

import contextlib
import ctypes
import sys
import types

import numpy as np

import concourse.bass as bass
import concourse.mybir as mybir
from concourse.tile import TileContext
from concourse.bass_utils import run_bass_kernel_spmd

# ---------------------------------------------------------------- constants
N_NODES = 50000
N_EDGES = 500000
HID = 128
P = 128
NCORES = 8

N_PAD = 50048            # 391 * 128
NTILES_N = N_PAD // P    # 391 node tiles
E_SHARD = N_EDGES // NCORES        # 62500
COLS = 489               # edge tile-columns per core
E_SHARD_PAD = COLS * P   # 62592
GK = 12                  # tile-columns per store group

F32 = mybir.dt.float32
BF16 = mybir.dt.bfloat16
I32 = mybir.dt.int32


# ------------------------------------------------------- walrus wait-limit fix
def _split_excess_waits(nc):
    """This toolchain's walrus rejects instructions with >1 sync-wait.
    Hoist extra waits onto standalone EventSemaphore instructions placed
    immediately before the offender on the same engine."""
    ctr = 0
    for fn in nc.m.functions:
        for bb in fn.blocks:
            new_insts = []
            for inst in bb.instructions:
                si = inst.sync_info
                if si is not None and si.on_wait and len(si.on_wait) > 1:
                    waits = list(si.on_wait)
                    si.on_wait.clear()
                    si.on_wait.append(waits[0])
                    for w in waits[1:]:
                        es = mybir.InstEventSemaphore(
                            name=f"waitsplit-{ctr}",
                            opcode="EventSemaphore",
                            engine=inst.engine,
                            ins=[],
                            outs=[],
                            sync_info=mybir.SyncInfo(on_wait=[w], on_update=[]),
                        )
                        ctr += 1
                        new_insts.append(es)
                new_insts.append(inst)
            bb.instructions.clear()
            bb.instructions.extend(new_insts)
    return ctr


# ----------------------------------------------------- NTFF profile hook shim
def _install_ntff_shim():
    """antenv.axon_hooks is missing from this image; provide it so
    run_bass_kernel_spmd(trace=True) can capture NTFF profiles."""
    if "antenv.axon_hooks" in sys.modules:
        return
    state = {"hook": None, "built": False}

    def _build():
        try:
            lib = ctypes.CDLL("/opt/axon/libaxon_pjrt.so")
        except OSError:
            return None
        if not hasattr(lib, "axon_start_nrt_profile"):
            return None
        lib.axon_start_nrt_profile.argtypes = [
            ctypes.POINTER(ctypes.c_int64),
            ctypes.c_size_t,
        ]
        lib.axon_start_nrt_profile.restype = ctypes.c_int64
        lib.axon_stop_nrt_profile.argtypes = [ctypes.c_char_p]
        lib.axon_stop_nrt_profile.restype = ctypes.c_int64

        @contextlib.contextmanager
        def _hook(output_dir, device_ids):
            import jax

            jax.devices()
            if device_ids:
                ids = (ctypes.c_int64 * len(device_ids))(*device_ids)
                rc = lib.axon_start_nrt_profile(ids, len(device_ids))
            else:
                rc = lib.axon_start_nrt_profile(None, 0)
            if rc != 0:
                raise RuntimeError(f"axon_start_nrt_profile rc={rc}")
            try:
                yield
            finally:
                n = lib.axon_stop_nrt_profile(str(output_dir).encode())
                print(f"ntff profile: {n} file(s) -> {output_dir}", file=sys.stderr)

        return _hook

    def get_axon_ntff_profile_hook():
        if not state["built"]:
            state["hook"] = _build()
            state["built"] = True
        return state["hook"]

    def set_axon_ntff_profile_hook(h):
        state["hook"] = h
        state["built"] = True

    mod = types.ModuleType("antenv.axon_hooks")
    mod.get_axon_ntff_profile_hook = get_axon_ntff_profile_hook
    mod.set_axon_ntff_profile_hook = set_axon_ntff_profile_hook
    sys.modules["antenv.axon_hooks"] = mod


_install_ntff_shim()


# ------------------------------------------------------------- device program
_NC_CACHE = {}


def _build_nc():
    if "nc" in _NC_CACHE:
        return _NC_CACHE["nc"]

    nc = bass.Bass()

    xT = nc.declare_dram_parameter("xT", [P, N_PAD], BF16, isOutput=False)
    WT = nc.declare_dram_parameter("WT", [HID, HID], BF16, isOutput=False)
    bbc = nc.declare_dram_parameter("bbc", [P, 4 * HID], F32, isOutput=False)
    v_in = nc.declare_dram_parameter("v", [E_SHARD_PAD, 3 * HID], F32, isOutput=False)
    u_lay = nc.declare_dram_parameter("u", [P, COLS * 3], F32, isOutput=False)
    src_l = nc.declare_dram_parameter("src", [P, COLS], I32, isOutput=False)
    dst_l = nc.declare_dram_parameter("dst", [P, COLS], I32, isOutput=False)
    o_out = nc.declare_dram_parameter("out", [E_SHARD_PAD, 3 * HID], F32, isOutput=True)

    # partition-major edge grid: edge = p*COLS + j
    v2 = v_in.rearrange("(p j) c -> p j c", p=P)    # [128, COLS, 384]
    o2 = o_out.rearrange("(p j) c -> p j c", p=P)

    with TileContext(nc) as tc:
        with (
            tc.tile_pool(name="hdram", bufs=1, space="DRAM") as hpool,
            tc.tile_pool(name="const", bufs=1) as cpool,
            tc.tile_pool(name="gemm", bufs=2) as gpool,
            tc.tile_pool(name="gpsum", bufs=4, space="PSUM") as pspool,
            tc.tile_pool(name="gath", bufs=3) as ga_pool,
            tc.tile_pool(name="vq", bufs=2) as vq_pool,
        ):
            h_dram = hpool.tile([N_PAD, HID], BF16)

            # ---- constants
            WT_s = cpool.tile([HID, HID], BF16, tag="wt")
            nc.sync.dma_start(out=WT_s[:], in_=WT[:])
            bbc_s = cpool.tile([P, 4 * HID], F32, tag="bbc")
            nc.sync.dma_start(out=bbc_s[:], in_=bbc[:])
            u_s = cpool.tile([P, COLS * 3], F32, tag="u")
            nc.sync.dma_start(out=u_s[:], in_=u_lay[:])
            src_s = cpool.tile([P, COLS], I32, tag="src")
            nc.sync.dma_start(out=src_s[:], in_=src_l[:])
            dst_s = cpool.tile([P, COLS], I32, tag="dst")
            nc.sync.dma_start(out=dst_s[:], in_=dst_l[:])

            # ---- phase 1: h = xT.T @ WT + b   (scales pre-folded on host;
            #      bf16 inputs -> full-rate PE, f32 PSUM; bias added on DVE).
            #      DMAs batched: x in 8192-node chunks, h stored 8 tiles at
            #      a time — the Sync sequencer costs ~600ns per dma_start.
            XCH = 64            # node-tiles per x-load chunk
            HB = 8              # node-tiles per h-store DMA
            h3 = h_dram[:].rearrange("(k p) c -> p k c", p=P)  # [128,391,128]
            for t0 in range(0, NTILES_N, XCH):
                tn = min(XCH, NTILES_N - t0)
                x_ch = gpool.tile([P, XCH * P], BF16, tag="xch")
                nc.sync.dma_start(
                    out=x_ch[:, :tn * P],
                    in_=xT[:, t0 * P:(t0 + tn) * P])
                for tb in range(0, tn, HB):
                    bn = min(HB, tn - tb)
                    h_sb = gpool.tile([P, HB * HID], BF16, tag="hsb")
                    for tq in range(0, bn, 4):
                        qn = min(4, bn - tq)
                        h_ps = pspool.tile([P, 4 * HID], F32, tag="hps")
                        for ti in range(qn):
                            t = tb + tq + ti
                            nc.tensor.matmul(
                                h_ps[:, ti * HID:(ti + 1) * HID],
                                lhsT=x_ch[:, t * P:(t + 1) * P],
                                rhs=WT_s[:], start=True, stop=True)
                        nc.vector.tensor_add(
                            out=h_sb[:, (tq) * HID:(tq + qn) * HID],
                            in0=h_ps[:, :qn * HID],
                            in1=bbc_s[:, :qn * HID])
                    nc.sync.dma_start(
                        out=h3[:, t0 + tb:t0 + tb + bn, :],
                        in_=h_sb[:, :bn * HID].rearrange(
                            "p (k c) -> p k c", c=HID))

            # ---- phase 2: per-edge message + residual
            j0 = 0
            while j0 < COLS:
                gk = min(GK, COLS - j0)
                hs_g = ga_pool.tile([P, GK * HID], BF16, tag="hs")
                hd_g = ga_pool.tile([P, GK * HID], BF16, tag="hd")
                for jl in range(gk):
                    j = j0 + jl
                    nc.gpsimd.indirect_dma_start(
                        out=hs_g[:, jl * HID:(jl + 1) * HID], out_offset=None,
                        in_=h_dram[:],
                        in_offset=bass.IndirectOffsetOnAxis(
                            ap=src_s[:, j:j + 1], axis=0))
                    nc.gpsimd.indirect_dma_start(
                        out=hd_g[:, jl * HID:(jl + 1) * HID], out_offset=None,
                        in_=h_dram[:],
                        in_offset=bass.IndirectOffsetOnAxis(
                            ap=dst_s[:, j:j + 1], axis=0))
                phi_g = ga_pool.tile([P, GK * HID], F32, tag="phi")
                nc.vector.tensor_tensor(
                    out=phi_g[:, :gk * HID], in0=hs_g[:, :gk * HID],
                    in1=hd_g[:, :gk * HID], op=mybir.AluOpType.subtract)

                v_g = vq_pool.tile([P, GK * 3 * HID], F32, tag="vg")
                nc.sync.dma_start(
                    out=v_g[:, :gk * 3 * HID].rearrange("p (j c) -> p j c", c=3 * HID),
                    in_=v2[:, j0:j0 + gk, :])

                q_g = vq_pool.tile([P, GK * 3 * HID], F32, tag="qg")
                # q[p, j, i, c] = phi[p, j, c] * u[p, (j0+j)*3 + i]
                pa = phi_g[:]
                phi_b = bass.AP(pa.tensor, pa.offset,
                                [pa.ap[0], [HID, gk], [0, 3], [1, HID]])
                ua = u_s[:, j0 * 3:(j0 + gk) * 3]
                u_b = bass.AP(ua.tensor, ua.offset,
                              [ua.ap[0], [3, gk], [1, 3], [0, HID]])
                qa = q_g[:]
                q_b = bass.AP(qa.tensor, qa.offset,
                              [qa.ap[0], [3 * HID, gk], [HID, 3], [1, HID]])
                nc.vector.tensor_tensor(out=q_b, in0=phi_b, in1=u_b,
                                        op=mybir.AluOpType.mult)
                # s = q + v  (DVE), then *0.5 (ScalarE), store
                nc.vector.tensor_add(
                    out=v_g[:, :gk * 3 * HID], in0=v_g[:, :gk * 3 * HID],
                    in1=q_g[:, :gk * 3 * HID])
                nc.scalar.mul(v_g[:, :gk * 3 * HID], v_g[:, :gk * 3 * HID], 0.5)
                nc.sync.dma_start(
                    out=o2[:, j0:j0 + gk, :],
                    in_=v_g[:, :gk * 3 * HID].rearrange(
                        "p (j c) -> p j c", c=3 * HID))
                j0 += gk

    _split_excess_waits(nc)
    _NC_CACHE["nc"] = nc
    return nc


# ------------------------------------------------------------------ host side
def _to_bf16(a):
    import ml_dtypes

    return a.astype(ml_dtypes.bfloat16)


def _prep_core_inputs(xT_np, WT_np, brow_np, v, u, src, dst, c):
    lo = c * E_SHARD
    hi = lo + E_SHARD

    v_sh = np.zeros((E_SHARD_PAD, 3 * HID), dtype=np.float32)
    v_sh[:E_SHARD] = v[lo:hi].reshape(E_SHARD, 3 * HID)

    u_sh = np.zeros((E_SHARD_PAD, 3), dtype=np.float32)
    u_sh[:E_SHARD] = u[lo:hi]
    u_lay = np.ascontiguousarray(u_sh.reshape(P, COLS * 3))

    def lay_idx(a):
        a_sh = np.zeros((E_SHARD_PAD,), dtype=np.int32)
        a_sh[:E_SHARD] = a[lo:hi].astype(np.int32)
        return np.ascontiguousarray(a_sh.reshape(P, COLS))

    return {
        "xT": xT_np,
        "WT": WT_np,
        "bbc": brow_np,
        "v": v_sh,
        "u": u_lay,
        "src": lay_idx(src),
        "dst": lay_idx(dst),
    }


def kernel(x, v, u, W, b, src, dst, _trace=False):
    x = np.asarray(x, dtype=np.float32)
    v = np.asarray(v, dtype=np.float32)
    u = np.asarray(u, dtype=np.float32)
    W = np.asarray(W, dtype=np.float32)
    b = np.asarray(b, dtype=np.float32)
    src = np.asarray(src)
    dst = np.asarray(dst)

    x_pad = np.zeros((N_PAD, HID), dtype=np.float32)
    x_pad[:N_NODES] = x
    xT_np = _to_bf16(np.ascontiguousarray(x_pad.T))       # [128, N_PAD] bf16
    WT_np = _to_bf16(np.ascontiguousarray((W / 256.0).T))  # fold 1/256
    brow_np = np.ascontiguousarray(
        np.tile((b / 256.0).astype(np.float32), (P, 4)))

    nc = _build_nc()
    in_maps = [
        _prep_core_inputs(xT_np, WT_np, brow_np, v, u, src, dst, c)
        for c in range(NCORES)
    ]
    res = run_bass_kernel_spmd(nc, in_maps, list(range(NCORES)), trace=_trace)

    out = np.empty((N_EDGES, 3, HID), dtype=np.float32)
    for c in range(NCORES):
        shard = res.results[c]["out"][:E_SHARD]
        out[c * E_SHARD:(c + 1) * E_SHARD] = shard.reshape(E_SHARD, 3, HID)
    if _trace:
        kernel.last_exec_time_ns = res.exec_time_ns
        kernel.last_results = res
    return out



# revision 5
# speedup vs baseline: 2.7010x; 2.7010x over previous
"""Trainium2 Bass kernel for nn_EquivBlock (GNN message passing).

Math (reference):
    h   = (x @ W.T + b) / 256            # [N, H] node projection
    phi = h[src] - h[dst]                # [E, H] per-edge message
    out = (v + u[:, :, None] * phi[:, None, :]) / 2

Key identity: h is affine in x, so the bias cancels in the difference:
    phi = (x[src] - x[dst]) @ (W/256).T
The host replicates x rows per edge (a pure gather/relayout of the input,
per the sharding hint "replicate node features x"), so the device sees
only contiguous streams -- no indirect DMA, no h table, no Q7 descriptor
generation storm (which was ~1.4ms of the 1.9ms baseline).

Device mapping (8 NeuronCores, SPMD, edges sharded; per core 62500 edges
padded to 489 tile-columns x 128):
    - per 128-edge tile: phi [128e, 128f] = xsT_tile.T @ WTp + xdT_tile.T
      @ (-WTp) accumulated in PSUM (bf16 inputs, f32 PSUM, one full 2KB
      bank per phi tile -- PE-write/DVE-read of the same bank is fatal),
    - DVE tensor_scalar_mul x3 per tile: q_i = phi * u[:, i] (per-
      partition scalar AP), PSUM -> SBUF,
    - per group of GK tiles: HWDGE v load, GpSimd add (s = q + v),
      ScalarE *0.5, store from the ACT HWDGE ring.

Host side only shards/pads/relayouts inputs and folds the constant
1/256 into W.
"""

import contextlib
import ctypes
import sys
import types

import numpy as np

import concourse.bass as bass
import concourse.mybir as mybir
from concourse.tile import TileContext
from concourse.bass_utils import run_bass_kernel_spmd

# ---------------------------------------------------------------- constants
N_NODES = 50000
N_EDGES = 500000
HID = 128
P = 128
NCORES = 8

E_SHARD = N_EDGES // NCORES        # 62500
COLS = 489                         # edge tile-columns per core
E_SHARD_PAD = COLS * P             # 62592
GK = 16                            # tile-columns per store group

F32 = mybir.dt.float32
BF16 = mybir.dt.bfloat16


# ------------------------------------------------------- walrus wait-limit fix
def _split_excess_waits(nc):
    """This toolchain's walrus rejects instructions with >1 sync-wait.
    Hoist extra waits onto standalone EventSemaphore instructions placed
    immediately before the offender on the same engine."""
    ctr = 0
    for fn in nc.m.functions:
        for bb in fn.blocks:
            new_insts = []
            for inst in bb.instructions:
                si = inst.sync_info
                if si is not None and si.on_wait and len(si.on_wait) > 1:
                    waits = list(si.on_wait)
                    si.on_wait.clear()
                    si.on_wait.append(waits[0])
                    for w in waits[1:]:
                        es = mybir.InstEventSemaphore(
                            name=f"waitsplit-{ctr}",
                            opcode="EventSemaphore",
                            engine=inst.engine,
                            ins=[],
                            outs=[],
                            sync_info=mybir.SyncInfo(on_wait=[w], on_update=[]),
                        )
                        ctr += 1
                        new_insts.append(es)
                new_insts.append(inst)
            bb.instructions.clear()
            bb.instructions.extend(new_insts)
    return ctr


# ----------------------------------------------------- NTFF profile hook shim
def _install_ntff_shim():
    """antenv.axon_hooks is missing from this image; provide it so
    run_bass_kernel_spmd(trace=True) can capture NTFF profiles."""
    if "antenv.axon_hooks" in sys.modules:
        return
    state = {"hook": None, "built": False}

    def _build():
        try:
            lib = ctypes.CDLL("/opt/axon/libaxon_pjrt.so")
        except OSError:
            return None
        if not hasattr(lib, "axon_start_nrt_profile"):
            return None
        lib.axon_start_nrt_profile.argtypes = [
            ctypes.POINTER(ctypes.c_int64),
            ctypes.c_size_t,
        ]
        lib.axon_start_nrt_profile.restype = ctypes.c_int64
        lib.axon_stop_nrt_profile.argtypes = [ctypes.c_char_p]
        lib.axon_stop_nrt_profile.restype = ctypes.c_int64

        @contextlib.contextmanager
        def _hook(output_dir, device_ids):
            import jax

            jax.devices()
            if device_ids:
                ids = (ctypes.c_int64 * len(device_ids))(*device_ids)
                rc = lib.axon_start_nrt_profile(ids, len(device_ids))
            else:
                rc = lib.axon_start_nrt_profile(None, 0)
            if rc != 0:
                raise RuntimeError(f"axon_start_nrt_profile rc={rc}")
            try:
                yield
            finally:
                n = lib.axon_stop_nrt_profile(str(output_dir).encode())
                print(f"ntff profile: {n} file(s) -> {output_dir}", file=sys.stderr)

        return _hook

    def get_axon_ntff_profile_hook():
        if not state["built"]:
            state["hook"] = _build()
            state["built"] = True
        return state["hook"]

    def set_axon_ntff_profile_hook(h):
        state["hook"] = h
        state["built"] = True

    mod = types.ModuleType("antenv.axon_hooks")
    mod.get_axon_ntff_profile_hook = get_axon_ntff_profile_hook
    mod.set_axon_ntff_profile_hook = set_axon_ntff_profile_hook
    sys.modules["antenv.axon_hooks"] = mod


_install_ntff_shim()


# ------------------------------------------------------------- device program
_NC_CACHE = {}


def _build_nc():
    if "nc" in _NC_CACHE:
        return _NC_CACHE["nc"]

    nc = bass.Bass()

    # xsT/xdT: x rows gathered per edge, transposed: column j = t*128 + p
    # holds x[src[p*COLS + t]] (t-major so tile t's 128 edges are one
    # contiguous [128, 128] slab feeding the PE as lhsT).
    xsT = nc.declare_dram_parameter("xsT", [P, E_SHARD_PAD], BF16, isOutput=False)
    xdT = nc.declare_dram_parameter("xdT", [P, E_SHARD_PAD], BF16, isOutput=False)
    WTp = nc.declare_dram_parameter("WTp", [HID, HID], BF16, isOutput=False)
    WTn = nc.declare_dram_parameter("WTn", [HID, HID], BF16, isOutput=False)
    v_in = nc.declare_dram_parameter("v", [E_SHARD_PAD, 3 * HID], F32, isOutput=False)
    u_lay = nc.declare_dram_parameter("u", [P, COLS * 3], F32, isOutput=False)
    o_out = nc.declare_dram_parameter("out", [E_SHARD_PAD, 3 * HID], F32, isOutput=True)

    # partition-major edge grid: edge = p*COLS + t
    v2 = v_in.rearrange("(p j) c -> p j c", p=P)    # [128, COLS, 384]
    o2 = o_out.rearrange("(p j) c -> p j c", p=P)

    with TileContext(nc) as tc:
        with (
            tc.tile_pool(name="const", bufs=1) as cpool,
            tc.tile_pool(name="xs", bufs=3) as xs_pool,
            tc.tile_pool(name="xd", bufs=3) as xd_pool,
            tc.tile_pool(name="phips", bufs=8, space="PSUM") as pspool,
            tc.tile_pool(name="qg", bufs=2) as q_pool,
            tc.tile_pool(name="vg", bufs=3) as v_pool,
        ):
            # ---- constants
            WTp_s = cpool.tile([HID, HID], BF16, tag="wtp")
            nc.sync.dma_start(out=WTp_s[:], in_=WTp[:])
            WTn_s = cpool.tile([HID, HID], BF16, tag="wtn")
            nc.sync.dma_start(out=WTn_s[:], in_=WTn[:])
            u_s = cpool.tile([P, COLS * 3], F32, tag="u")
            nc.sync.dma_start(out=u_s[:], in_=u_lay[:])

            t0 = 0
            while t0 < COLS:
                gk = min(GK, COLS - t0)
                xs_g = xs_pool.tile([P, GK * P], BF16, tag="xs")
                xd_g = xd_pool.tile([P, GK * P], BF16, tag="xd")
                nc.sync.dma_start(
                    out=xs_g[:, :gk * P], in_=xsT[:, t0 * P:(t0 + gk) * P])
                nc.sync.dma_start(
                    out=xd_g[:, :gk * P], in_=xdT[:, t0 * P:(t0 + gk) * P])

                v_g = v_pool.tile([P, GK * 3 * HID], F32, tag="vg")
                nc.sync.dma_start(
                    out=v_g[:, :gk * 3 * HID].rearrange(
                        "p (j c) -> p j c", c=3 * HID),
                    in_=v2[:, t0:t0 + gk, :])

                q_g = q_pool.tile([P, GK * 3 * HID], F32, tag="qg")
                for tl in range(gk):
                    t = t0 + tl
                    phi_bank = pspool.tile([P, 512], F32, tag="phi")
                    phi_ps = phi_bank[:, :HID]
                    nc.tensor.matmul(
                        phi_ps,
                        lhsT=xs_g[:, tl * P:(tl + 1) * P],
                        rhs=WTp_s[:], start=True, stop=False)
                    nc.tensor.matmul(
                        phi_ps,
                        lhsT=xd_g[:, tl * P:(tl + 1) * P],
                        rhs=WTn_s[:], start=False, stop=True)
                    # q[p, i*HID:(i+1)*HID] = phi[p, :] * u[p, t*3 + i]
                    for i in range(3):
                        nc.vector.tensor_scalar_mul(
                            q_g[:, tl * 3 * HID + i * HID:
                                tl * 3 * HID + (i + 1) * HID],
                            phi_ps,
                            u_s[:, t * 3 + i:t * 3 + i + 1])

                # s = q + v on GpSimd (DVE is busy with the qmults)
                nc.gpsimd.tensor_add(
                    out=v_g[:, :gk * 3 * HID],
                    in0=v_g[:, :gk * 3 * HID],
                    in1=q_g[:, :gk * 3 * HID])
                # out = s * 0.5, then store (ACT HWDGE ring keeps the sync
                # ring free for the loads)
                nc.scalar.mul(v_g[:, :gk * 3 * HID], v_g[:, :gk * 3 * HID], 0.5)
                nc.scalar.dma_start(
                    out=o2[:, t0:t0 + gk, :],
                    in_=v_g[:, :gk * 3 * HID].rearrange(
                        "p (j c) -> p j c", c=3 * HID))
                t0 += gk

    _split_excess_waits(nc)
    _NC_CACHE["nc"] = nc
    return nc


# ------------------------------------------------------------------ host side
def _to_bf16(a):
    import ml_dtypes

    return a.astype(ml_dtypes.bfloat16)


def _prep_core_inputs(xT_bf, WTp_np, WTn_np, v, u, src, dst, c):
    lo = c * E_SHARD
    hi = lo + E_SHARD

    v_sh = np.zeros((E_SHARD_PAD, 3 * HID), dtype=np.float32)
    v_sh[:E_SHARD] = v[lo:hi].reshape(E_SHARD, 3 * HID)

    u_sh = np.zeros((E_SHARD_PAD, 3), dtype=np.float32)
    u_sh[:E_SHARD] = u[lo:hi]
    u_lay = np.ascontiguousarray(u_sh.reshape(P, COLS * 3))

    def lay_idx(a):
        a_sh = np.zeros((E_SHARD_PAD,), dtype=np.int64)
        a_sh[:E_SHARD] = a[lo:hi]
        return a_sh.reshape(P, COLS)

    # t-major gather order: xsT column t*128 + p = x[src[p*COLS + t]]
    src_tmaj = lay_idx(src).T.reshape(-1)
    dst_tmaj = lay_idx(dst).T.reshape(-1)
    xs_np = np.ascontiguousarray(xT_bf[:, src_tmaj])
    xd_np = np.ascontiguousarray(xT_bf[:, dst_tmaj])

    return {
        "xsT": xs_np,
        "xdT": xd_np,
        "WTp": WTp_np,
        "WTn": WTn_np,
        "v": v_sh,
        "u": u_lay,
    }


def kernel(x, v, u, W, b, src, dst, _trace=False):
    x = np.asarray(x, dtype=np.float32)
    v = np.asarray(v, dtype=np.float32)
    u = np.asarray(u, dtype=np.float32)
    W = np.asarray(W, dtype=np.float32)
    src = np.asarray(src)
    dst = np.asarray(dst)

    xT_bf = _to_bf16(np.ascontiguousarray(x.T))            # [128, N] bf16
    WT_scaled = np.ascontiguousarray((W / 256.0).T)
    WTp_np = _to_bf16(WT_scaled)
    WTn_np = _to_bf16(-WT_scaled)

    nc = _build_nc()
    in_maps = [
        _prep_core_inputs(xT_bf, WTp_np, WTn_np, v, u, src, dst, c)
        for c in range(NCORES)
    ]
    res = run_bass_kernel_spmd(nc, in_maps, list(range(NCORES)), trace=_trace)

    out = np.empty((N_EDGES, 3, HID), dtype=np.float32)
    for c in range(NCORES):
        shard = res.results[c]["out"][:E_SHARD]
        out[c * E_SHARD:(c + 1) * E_SHARD] = shard.reshape(E_SHARD, 3, HID)
    if _trace:
        kernel.last_exec_time_ns = res.exec_time_ns
        kernel.last_results = res
    return out
